# revision 8
# baseline (speedup 1.0000x reference)
"""Trainium2 Bass kernel for nn_ActorCom (GRU autoregressive sampler).

Contract: kernel(**inputs) takes FULL unsharded inputs (keyed as in
reference.setup_inputs()) and returns the FULL output tuple
(message, logits, posterior, masks) matching reference.reference(**inputs).

Strategy (pure data parallel over 8 NeuronCores, 4096 batch rows each):
  - jax.random.categorical(key, logit) == argmax(logit + gumbel(key)); the keys
    are compile-time constants (jax.random.key(42)), so the gumbel noise is
    input-independent and precomputed on host (CPU jax, bit-exact with the
    reference) and streamed to the device.
  - The embedding gather xa = (embed_table @ Wx + bx)[token] has only 28
    possible rows; it is realized as a one-hot (28,B) matmul fused into the
    gate matmul via a stacked contraction of K=92 = 64 (h) + 28 (one-hot).
  - State layout: S (92, 4096) = [hT (64,B); one-hotT (28,B)].  Gate
    pre-activations come out gate-major: one K=92 M=128 N=512 fp32 matmul per
    512-batch tile for [z|r], one for [hh|xh] (block-diagonal lhsT).
  - Sampling runs batch-major: logits (128,26) per 128-row group via
    lhsT=S-slice matmuls; argmax via segmented max + is_equal one-hot; softmax
    with segmented reductions; token/done updates with predicated copies.
  - The sampled tokens (128,32) are transposed via the PE, linearized by DMA
    to a (1,4096) row, partition-broadcast to (28,4096) and compared against
    an iota column to rebuild one-hotT for the next step.
All matmuls are standard fp32 (fp32r measured 1.9e-4 scale-relative error on
HW - too coarse for stable argmax; fp32 measured 2.7e-7).
"""
import numpy as np

import concourse.bass as bass
import concourse.mybir as mybir
import concourse.tile as tile
from concourse import bacc
from concourse.bass_utils import run_bass_kernel_spmd

F32 = mybir.dt.float32
BF16 = mybir.dt.bfloat16
I32 = mybir.dt.int32
AF = mybir.ActivationFunctionType
OP = mybir.AluOpType
AX = mybir.AxisListType

VOCAB = 25
END, START, PAD = 25, 26, 27
EMBED, HID, OUT, MAXLEN = 3, 64, 26, 25
BATCH, FEAT = 32768, 512
NCORES = 8
B = BATCH // NCORES          # 4096 per core
M = B // 128                 # 32 m-groups of 128 rows
NT = B // 512                # 8 n-tiles of 512
KG = HID + PAD + 1           # 92 = 64 + 28 stacked contraction
VW = M * OUT                 # 832 = 32 groups * 26


def _build_nc():
    nc = bacc.Bacc("TRN2", target_bir_lowering=False, debug=False,
                   num_devices=NCORES)

    embT = nc.dram_tensor("embT", [FEAT, B], F32, kind="ExternalInput")
    gum = nc.dram_tensor("gum", [MAXLEN, 128, VW], F32, kind="ExternalInput")
    wzr = nc.dram_tensor("wzr", [KG, 128], F32, kind="ExternalInput")
    whx = nc.dram_tensor("whx", [KG, 128], F32, kind="ExternalInput")
    w_in = nc.dram_tensor("w_in", [FEAT, HID], F32, kind="ExternalInput")
    wout = nc.dram_tensor("wout", [HID, OUT], F32, kind="ExternalInput")
    bout = nc.dram_tensor("bout", [1, OUT], F32, kind="ExternalInput")
    ident = nc.dram_tensor("ident", [128, 128], F32, kind="ExternalInput")

    o_toks = nc.dram_tensor("o_toks", [MAXLEN, 128, M], I32, kind="ExternalOutput")
    o_masks = nc.dram_tensor("o_masks", [MAXLEN, 128, M], I32, kind="ExternalOutput")
    o_logp = nc.dram_tensor("o_logp", [128, M], F32, kind="ExternalOutput")
    o_post = nc.dram_tensor("o_post", [MAXLEN, 128, VW], F32, kind="ExternalOutput")

    with tile.TileContext(nc) as tc:
        with (
            tc.tile_pool(name="wp", bufs=1) as wp,
            tc.tile_pool(name="st", bufs=1) as st,
            tc.tile_pool(name="wk", bufs=2) as wk,
            tc.tile_pool(name="wk1", bufs=1) as wk1,
            tc.tile_pool(name="ps", bufs=2, space="PSUM") as ps,
        ):
            # ---- persistent weights / constants
            wzr_s = wp.tile([KG, 128], F32, tag="wzr")
            whx_s = wp.tile([KG, 128], F32, tag="whx")
            wout_s = wp.tile([HID, OUT], F32, tag="wout")
            bout_s = wp.tile([1, OUT], F32, tag="bout")
            id_s = wp.tile([128, 128], F32, tag="ident")
            nc.sync.dma_start(wzr_s[:], wzr[:])
            nc.sync.dma_start(whx_s[:], whx[:])
            nc.sync.dma_start(wout_s[:], wout[:])
            nc.sync.dma_start(bout_s[:], bout[:])
            nc.sync.dma_start(id_s[:], ident[:])

            c26 = wp.tile([128, OUT], F32, tag="c26")
            nc.gpsimd.iota(c26[:], pattern=[[1, OUT]], base=0,
                           channel_multiplier=0,
                           allow_small_or_imprecise_dtypes=True)
            c28 = wp.tile([PAD + 1, 1], F32, tag="c28")
            nc.gpsimd.iota(c28[:], pattern=[[0, 1]], base=0,
                           channel_multiplier=1,
                           allow_small_or_imprecise_dtypes=True)
            c27t = wp.tile([128, 1], F32, tag="c27t")
            nc.vector.memset(c27t[:], float(PAD))
            ones = wp.tile([128, 1], F32, tag="ones")
            nc.vector.memset(ones[:], 1.0)

            # ---- persistent state
            S = st.tile([KG, B], F32, tag="S")
            done_i = st.tile([128, M], I32, tag="done")
            acc = st.tile([128, M], F32, tag="acc")
            nc.vector.memset(done_i[:], 0)
            nc.vector.memset(acc[:], 0.0)

            def build_onehot(tokrow_ap):
                """S[64:92] <- onehotT of the token row (1, B)."""
                tokb = wk1.tile([PAD + 1, B], F32, tag="tokb", name="tokb")
                nc.gpsimd.partition_broadcast(tokb[:], tokrow_ap)
                for half_ in range(2):
                    hsl_ = slice(half_ * 2048, (half_ + 1) * 2048)
                    nc.gpsimd.tensor_scalar(
                        out=S[HID:KG, hsl_], in0=tokb[:, hsl_],
                        scalar1=c28[:, 0:1], scalar2=None,
                        op0=OP.is_equal)

            # one-hot(START) for step 0
            tok0 = wp.tile([1, B], F32, tag="tok0")
            nc.vector.memset(tok0[:], float(START))
            build_onehot(tok0[:])

            # ---- h0 = silu(emb @ W_in) ; computed transposed into S[0:64]
            win_t = [wp.tile([128, HID], F32, tag=f"win{k}", name=f"win{k}")
                     for k in range(4)]
            for k in range(4):
                nc.sync.dma_start(win_t[k][:], w_in[k * 128:(k + 1) * 128, :])
            h0ps = [ps.tile([HID, 2048], F32, tag="ps", name=f"h0ps{h}")
                    for h in range(2)]
            for k in range(4):
                ek = wk1.tile([128, B], F32, tag="embk")
                nc.sync.dma_start(ek[:], embT[k * 128:(k + 1) * 128, :])
                for half in range(2):
                    for q in range(4):
                        nt = half * 4 + q
                        nc.tensor.matmul(
                            h0ps[half][:, q * 512:(q + 1) * 512],
                            win_t[k][:],
                            ek[:, nt * 512:(nt + 1) * 512],
                            start=(k == 0), stop=(k == 3))
            for half in range(2):
                hsl0 = slice(half * 2048, (half + 1) * 2048)
                sg0 = wk.tile([HID, 2048], F32, tag="sg0")
                nc.scalar.activation(sg0[:], h0ps[half][:], AF.Sigmoid)
                nc.vector.tensor_tensor(out=S[0:HID, hsl0], in0=h0ps[half][:],
                                        in1=sg0[:], op=OP.mult)

            # ---- autoregressive loop
            for t in range(MAXLEN):
                gt = wk.tile([128, VW], F32, tag="gum")
                nc.sync.dma_start(gt[:], gum[t])

                # gate phase: h <- GRU(onehot, h), in halves of 2048 batch
                for half in range(2):
                    hsl = slice(half * 2048, (half + 1) * 2048)
                    zr4 = ps.tile([128, 2048], F32, tag="ps")
                    hx4 = ps.tile([128, 2048], F32, tag="ps")
                    for q in range(4):
                        nt = half * 4 + q
                        sl = slice(nt * 512, (nt + 1) * 512)
                        nc.tensor.matmul(zr4[:, q * 512:(q + 1) * 512],
                                         wzr_s[:], S[:, sl],
                                         start=True, stop=True)
                        nc.tensor.matmul(hx4[:, q * 512:(q + 1) * 512],
                                         whx_s[:], S[:, sl],
                                         start=True, stop=True)
                    zrs = wk.tile([128, 2048], F32, tag="zrs")
                    nc.scalar.activation(zrs[:], zr4[:], AF.Sigmoid)
                    t1 = wk1.tile([HID, 2048], F32, tag="t1")
                    nc.vector.tensor_tensor(out=t1[:], in0=zrs[HID:128, :],
                                            in1=hx4[0:HID, :], op=OP.mult)
                    t2 = wk1.tile([HID, 2048], F32, tag="t2")
                    nc.vector.tensor_tensor(out=t2[:], in0=t1[:],
                                            in1=hx4[HID:128, :], op=OP.add)
                    nn_ = wk1.tile([HID, 2048], F32, tag="nn")
                    nc.scalar.activation(nn_[:], t2[:], AF.Tanh)
                    dd = wk1.tile([HID, 2048], F32, tag="dd")
                    nc.vector.tensor_tensor(out=dd[:], in0=S[0:HID, hsl],
                                            in1=nn_[:], op=OP.subtract)
                    ee = wk1.tile([HID, 2048], F32, tag="ee")
                    nc.gpsimd.tensor_tensor(out=ee[:], in0=dd[:],
                                            in1=zrs[0:HID, :], op=OP.mult)
                    nc.gpsimd.tensor_tensor(out=S[0:HID, hsl], in0=ee[:],
                                            in1=nn_[:], op=OP.add)

                # logit phase: 32 groups of 128 rows, 16 per psum tile
                L = wk1.tile([128, VW], F32, tag="L")
                for half in range(2):
                    lg4 = ps.tile([128, 2048], F32, tag="ps")
                    for g in range(16):
                        m = half * 16 + g
                        off = (g // 4) * 512 + (g % 4) * OUT
                        nc.tensor.matmul(
                            lg4[:, off:off + OUT],
                            S[0:HID, m * 128:(m + 1) * 128],
                            wout_s[:], start=True, stop=True)
                    for q in range(4):
                        dst = (half * 16 + q * 4) * OUT
                        nc.vector.tensor_copy(
                            L[:, dst:dst + 4 * OUT],
                            lg4[:, q * 512:q * 512 + 4 * OUT])

                L3 = L[:].rearrange("p (m v) -> p m v", v=OUT)
                Lg = wk1.tile([128, VW], F32, tag="Lg")
                Lg3 = Lg[:].rearrange("p (m v) -> p m v", v=OUT)
                nc.vector.tensor_tensor(out=Lg[:], in0=L[:], in1=gt[:], op=OP.add)

                mg = wk.tile([128, M], F32, tag="mg")
                nc.vector.tensor_reduce(mg[:], Lg3, axis=AX.X, op=OP.max)
                OH = wk1.tile([128, VW], F32, tag="OH")
                OH3 = OH[:].rearrange("p (m v) -> p m v", v=OUT)
                nc.vector.tensor_tensor(
                    out=OH3, in0=Lg3,
                    in1=mg[:][:, :, None].broadcast_to([128, M, OUT]),
                    op=OP.is_equal)

                # softmax of raw logits (max-subtracted like jax.nn.softmax)
                ms = wk1.tile([128, M], F32, tag="ms")
                nc.vector.tensor_reduce(ms[:], L3, axis=AX.X, op=OP.max)
                Lc = wk1.tile([128, VW], F32, tag="Lc")
                Lc3 = Lc[:].rearrange("p (m v) -> p m v", v=OUT)
                nc.vector.tensor_tensor(
                    out=Lc3, in0=L3,
                    in1=ms[:][:, :, None].broadcast_to([128, M, OUT]),
                    op=OP.subtract)
                E = wk1.tile([128, VW], F32, tag="E")
                nc.scalar.activation(E[:], Lc[:], AF.Exp)
                E3 = E[:].rearrange("p (m v) -> p m v", v=OUT)
                sE = wk1.tile([128, M], F32, tag="sE")
                nc.vector.tensor_reduce(sE[:], E3, axis=AX.X, op=OP.add)
                rcp = wk1.tile([128, M], F32, tag="rcp")
                nc.vector.reciprocal(rcp[:], sE[:])
                prob = wk.tile([128, VW], F32, tag="prob")
                prob3 = prob[:].rearrange("p (m v) -> p m v", v=OUT)
                nc.vector.tensor_tensor(
                    out=prob3, in0=E3,
                    in1=rcp[:][:, :, None].broadcast_to([128, M, OUT]),
                    op=OP.mult)
                nc.scalar.dma_start(o_post[t], prob[:])

                # p = prob[sample] ; sample = onehot . iota
                PM = wk1.tile([128, VW], F32, tag="PM")
                nc.vector.tensor_tensor(out=PM[:], in0=prob[:], in1=OH[:],
                                        op=OP.mult)
                PM3 = PM[:].rearrange("p (m v) -> p m v", v=OUT)
                p = wk.tile([128, M], F32, tag="p")
                nc.vector.tensor_reduce(p[:], PM3, axis=AX.X, op=OP.add)
                SM = wk1.tile([128, VW], F32, tag="SM")
                SM3 = SM[:].rearrange("p (m v) -> p m v", v=OUT)
                nc.vector.tensor_tensor(
                    out=SM3, in0=OH3,
                    in1=c26[:][:, None, :].broadcast_to([128, M, OUT]),
                    op=OP.mult)
                samp = wk.tile([128, M], F32, tag="samp")
                nc.vector.tensor_reduce(samp[:], SM3, axis=AX.X, op=OP.add)

                term_i = wk.tile([128, M], I32, tag="term")
                nc.vector.tensor_copy(term_i[:], OH3[:, :, END:END + 1])

                tokcol = wk.tile([128, M], F32, tag="tokcol")
                nc.vector.tensor_copy(tokcol[:], samp[:])
                nc.vector.copy_predicated(tokcol[:], done_i[:],
                                          c27t[:].broadcast_to([128, M]))
                pp = wk.tile([128, M], F32, tag="pp")
                nc.vector.tensor_copy(pp[:], p[:])
                nc.vector.copy_predicated(pp[:], done_i[:],
                                          ones[:].broadcast_to([128, M]))
                nc.vector.tensor_tensor(out=done_i[:], in0=done_i[:],
                                        in1=term_i[:], op=OP.max)

                lp = wk.tile([128, M], F32, tag="lp")
                nc.scalar.activation(lp[:], pp[:], AF.Ln)
                nc.vector.tensor_tensor(out=acc[:], in0=acc[:], in1=lp[:],
                                        op=OP.add)

                toks_i = wk.tile([128, M], I32, tag="toks_i")
                nc.vector.tensor_copy(toks_i[:], tokcol[:])
                nc.scalar.dma_start(o_toks[t], toks_i[:])
                nc.scalar.dma_start(o_masks[t], done_i[:])

                # bridge: rebuild one-hotT(token) for next step
                if t < MAXLEN - 1:
                    tp = ps.tile([M, 128], F32, tag="ps")
                    nc.tensor.transpose(tp[:], tokcol[:], id_s[:])
                    tokT = wk1.tile([M, 128], F32, tag="tokT")
                    nc.vector.tensor_copy(tokT[:], tp[:])
                    tokrow = wk1.tile([1, B], F32, tag="tokrow")
                    nc.sync.dma_start(tokrow[:], tokT[:])
                    build_onehot(tokrow[:])

            nc.sync.dma_start(o_logp[:], acc[:])

    nc.compile()
    return nc


_NC_CACHE = {}


def _get_nc():
    if "nc" not in _NC_CACHE:
        _NC_CACHE["nc"] = _build_nc()
    return _NC_CACHE["nc"]


def _host_prep(inputs):
    emb = np.ascontiguousarray(np.asarray(inputs["embedding"], np.float32))
    embed_table = np.asarray(inputs["embed_table"], np.float32)
    W_in = np.ascontiguousarray(np.asarray(inputs["W_in"], np.float32))
    b_in = np.asarray(inputs["b_in"], np.float32)
    Wx = np.asarray(inputs["Wx"], np.float32)
    Wh = np.asarray(inputs["Wh"], np.float32)
    bx = np.asarray(inputs["bx"], np.float32)
    bh = np.asarray(inputs["bh"], np.float32)
    W_out = np.ascontiguousarray(np.asarray(inputs["W_out"], np.float32))
    b_out = np.asarray(inputs["b_out"], np.float32)

    if np.abs(b_in).max() > 0 or np.abs(bh).max() > 0 or np.abs(b_out).max() > 0:
        # biases are folded assuming the zero-initialized reference setup;
        # b_in/bh/b_out nonzero would need extra bias adds in the kernel
        raise NotImplementedError("nonzero b_in/bh/b_out not supported")

    xa_table = (embed_table @ Wx + bx).astype(np.float32)   # (28, 192)

    wzr = np.zeros((KG, 128), np.float32)
    wzr[0:HID, :] = Wh[:, 0:128]
    wzr[HID:KG, :] = xa_table[:, 0:128]
    whx = np.zeros((KG, 128), np.float32)
    whx[0:HID, 0:HID] = Wh[:, 128:192]          # hh -> out partitions 0:64
    whx[HID:KG, HID:128] = xa_table[:, 128:192]  # xh -> out partitions 64:128

    # gumbel noise, bit-exact with jax.random.categorical on CPU
    import jax
    import jax.numpy as jnp
    cpu = jax.devices("cpu")[0]
    with jax.default_device(cpu):
        keys = jax.random.split(jax.random.key(42), MAXLEN)
        G = np.stack([
            np.asarray(jax.random.gumbel(keys[t], (BATCH, OUT), jnp.float32))
            for t in range(MAXLEN)])                      # (25, 32768, 26)

    ident = np.eye(128, dtype=np.float32)
    bout2 = b_out.reshape(1, OUT)

    in_maps = []
    for c in range(NCORES):
        off = c * B
        embT = np.ascontiguousarray(emb[off:off + B, :].T)          # (512, 4096)
        gc = G[:, off:off + B, :].reshape(MAXLEN, M, 128, OUT)
        gc = np.ascontiguousarray(gc.transpose(0, 2, 1, 3).reshape(MAXLEN, 128, VW))
        in_maps.append({
            "embT": embT, "gum": gc, "wzr": wzr, "whx": whx,
            "w_in": W_in, "wout": W_out, "bout": bout2, "ident": ident,
        })
    return in_maps


def _assemble(results):
    msg_parts, logp_parts, post_parts, mask_parts = [], [], [], []
    for c in range(NCORES):
        r = results[c]
        toks = r["o_toks"].transpose(0, 2, 1).reshape(MAXLEN, B)     # [t, b]
        masks = r["o_masks"].transpose(0, 2, 1).reshape(MAXLEN, B)
        logp = r["o_logp"].transpose(1, 0).reshape(B)
        post = (r["o_post"].reshape(MAXLEN, 128, M, OUT)
                .transpose(2, 1, 0, 3).reshape(B, MAXLEN, OUT))
        done_fin = masks[-1]
        final = (END + 2 * done_fin).astype(np.int32)
        msg = np.concatenate([
            np.full((B, 1), START, np.int32),
            toks.T.astype(np.int32),
            final[:, None]], axis=1)
        msg_parts.append(msg)
        logp_parts.append(logp.astype(np.float32))
        post_parts.append(post[:, :, None, :].astype(np.float32))
        mask_parts.append(masks.T[:, :, None].astype(np.int32))
    return (np.concatenate(msg_parts, 0), np.concatenate(logp_parts, 0),
            np.concatenate(post_parts, 0), np.concatenate(mask_parts, 0))


def kernel(**inputs):
    nc = _get_nc()
    in_maps = _host_prep(inputs)
    res = run_bass_kernel_spmd(nc, in_maps, list(range(NCORES)))
    return _assemble(res.results)


if __name__ == "__main__":
    d = np.load("/tmp/inputs.npz")
    inputs = {k: d[k] for k in d.files}
    out = kernel(**inputs)
    for name, a in zip(["message", "logits", "posterior", "masks"], out):
        print(name, a.shape, a.dtype)


# revision 14
# speedup vs baseline: 1.0238x; 1.0238x over previous
"""Trainium2 Bass kernel for nn_ActorCom (GRU autoregressive sampler).

Contract: kernel(**inputs) takes FULL unsharded inputs (keyed as in
reference.setup_inputs()) and returns the FULL output tuple
(message, logits, posterior, masks) matching reference.reference(**inputs).

Strategy (pure data parallel over 8 NeuronCores, 4096 batch rows each):
  - jax.random.categorical(key, logit) == argmax(logit + gumbel(key)); the keys
    are compile-time constants (jax.random.key(42)), so the gumbel noise is
    input-independent and precomputed on host (CPU jax, bit-exact with the
    reference) and streamed to the device.
  - The embedding gather xa = (embed_table @ Wx + bx)[token] has only 28
    possible rows; it is realized as a one-hot (28,B) matmul fused into the
    gate matmul via a stacked contraction of K=92 = 64 (h) + 28 (one-hot).
  - State layout: S (92, 4096) = [hT (64,B); one-hotT (28,B)].  Gate
    pre-activations come out gate-major: one K=92 M=128 N=512 fp32 matmul per
    512-batch tile for [z|r], one for [hh|xh] (block-diagonal lhsT).
  - Sampling runs batch-major: logits (128,26) per 128-row group via
    lhsT=S-slice matmuls; argmax via segmented max + is_equal one-hot; softmax
    with segmented reductions; token/done updates with predicated copies.
  - Bridge to the next step: sampled tokens (128,32) are PE-transposed,
    DMA-linearized to a (1,4096) bf16 row, expanded to (28,B) differences
    tok[b]-v via a K=2 bf16 matmul against [ones; -iota28], and compared
    against 0 to rebuild one-hotT (exact: all values are small integers).
All gate/logit matmuls are fp32 (fp32r measured 1.9e-4 scale-relative error
on HW - too coarse for stable argmax; fp32 measured 2.7e-7).
"""
import numpy as np

import concourse.bass as bass
import concourse.mybir as mybir
import concourse.tile as tile
from concourse import bacc
from concourse.bass_utils import run_bass_kernel_spmd

F32 = mybir.dt.float32
BF16 = mybir.dt.bfloat16
I32 = mybir.dt.int32
AF = mybir.ActivationFunctionType
OP = mybir.AluOpType
AX = mybir.AxisListType

VOCAB = 25
END, START, PAD = 25, 26, 27
EMBED, HID, OUT, MAXLEN = 3, 64, 26, 25
BATCH, FEAT = 32768, 512
NCORES = 8
B = BATCH // NCORES          # 4096 per core
M = B // 128                 # 32 m-groups of 128 rows
KG = HID + PAD + 1           # 92 = 64 + 28 stacked contraction
VW = M * OUT                 # 832 = 32 groups * 26
NP = B // 1024               # 4 n-tile pairs of 1024


def _build_nc():
    nc = bacc.Bacc("TRN2", target_bir_lowering=False, debug=False,
                   num_devices=NCORES)

    embT = nc.dram_tensor("embT", [FEAT, B], F32, kind="ExternalInput")
    gum = nc.dram_tensor("gum", [MAXLEN, 128, VW], F32, kind="ExternalInput")
    wzr = nc.dram_tensor("wzr", [KG, 128], F32, kind="ExternalInput")
    whx = nc.dram_tensor("whx", [KG, 128], F32, kind="ExternalInput")
    w_in = nc.dram_tensor("w_in", [FEAT, HID], F32, kind="ExternalInput")
    wout = nc.dram_tensor("wout", [HID, OUT], F32, kind="ExternalInput")
    wbr = nc.dram_tensor("wbr", [2, PAD + 1], F32, kind="ExternalInput")
    ident = nc.dram_tensor("ident", [128, 128], F32, kind="ExternalInput")

    o_toks = nc.dram_tensor("o_toks", [MAXLEN, 128, M], I32, kind="ExternalOutput")
    o_masks = nc.dram_tensor("o_masks", [MAXLEN, 128, M], I32, kind="ExternalOutput")
    o_logp = nc.dram_tensor("o_logp", [128, M], F32, kind="ExternalOutput")
    o_post = nc.dram_tensor("o_post", [MAXLEN, 128, VW], F32, kind="ExternalOutput")

    with tile.TileContext(nc) as tc:
        with (
            tc.tile_pool(name="wp", bufs=1) as wp,
            tc.tile_pool(name="st", bufs=1) as st,
            tc.tile_pool(name="wk", bufs=2) as wk,
            tc.tile_pool(name="wk1", bufs=1) as wk1,
            tc.tile_pool(name="gps", bufs=3, space="PSUM") as gps,
            tc.tile_pool(name="lps", bufs=2, space="PSUM") as lps,
        ):
            # ---- persistent weights / constants
            wzr_s = wp.tile([KG, 128], F32, tag="wzr")
            whx_s = wp.tile([KG, 128], F32, tag="whx")
            wout_s = wp.tile([HID, OUT], F32, tag="wout")
            id_s = wp.tile([128, 128], F32, tag="ident")
            wbr_f = wp.tile([2, PAD + 1], F32, tag="wbrf")
            wbr_s = wp.tile([2, PAD + 1], BF16, tag="wbr")
            nc.sync.dma_start(wzr_s[:], wzr[:])
            nc.sync.dma_start(whx_s[:], whx[:])
            nc.sync.dma_start(wout_s[:], wout[:])
            nc.sync.dma_start(id_s[:], ident[:])
            nc.sync.dma_start(wbr_f[:], wbr[:])
            nc.vector.tensor_copy(wbr_s[:], wbr_f[:])

            c26 = wp.tile([128, OUT], F32, tag="c26")
            nc.gpsimd.iota(c26[:], pattern=[[1, OUT]], base=0,
                           channel_multiplier=0,
                           allow_small_or_imprecise_dtypes=True)
            c27t = wp.tile([128, 1], F32, tag="c27t")
            nc.vector.memset(c27t[:], float(PAD))
            ones = wp.tile([128, 1], F32, tag="ones")
            nc.vector.memset(ones[:], 1.0)

            # bridge rhs: row 0 = token row (bf16, refreshed), row 1 = ones
            tokr2 = st.tile([2, B], BF16, tag="tokr2")
            nc.vector.memset(tokr2[0:2, :], 1.0)   # row 1 stays ones forever

            # ---- persistent state
            S = st.tile([KG, B], F32, tag="S")
            done_i = st.tile([128, M], I32, tag="done")
            acc = st.tile([128, M], F32, tag="acc")
            nc.vector.memset(done_i[:], 0)
            nc.vector.memset(acc[:], 0.0)

            def build_onehot(dve_iseq=True):
                """S[64:92] <- onehot from token row tokr2[0] via K=2 matmul."""
                for j in range(8):
                    sl = slice(j * 512, (j + 1) * 512)
                    dp = lps.tile([PAD + 1, 512], F32, tag="lps", name=f"dp{j}")
                    nc.tensor.matmul(dp[:], wbr_s[:], tokr2[:, sl],
                                     start=True, stop=True)
                    eng = nc.vector if (j % 2 == 0) else nc.gpsimd
                    if dve_iseq:
                        eng = nc.vector
                    nc.vector.tensor_scalar(
                        out=S[HID:KG, sl], in0=dp[:], scalar1=0.0,
                        scalar2=None, op0=OP.is_equal)

            # one-hot(START) for step 0
            nc.vector.memset(tokr2[0:1, :], float(START))
            build_onehot()

            # ---- h0 = silu(emb @ W_in) ; computed transposed into S[0:64]
            win_t = [wp.tile([128, HID], F32, tag=f"win{k}", name=f"win{k}")
                     for k in range(4)]
            for k in range(4):
                nc.sync.dma_start(win_t[k][:], w_in[k * 128:(k + 1) * 128, :])
            for pg in range(2):          # pair-groups of 2048 batch cols
                csl = slice(pg * 2048, (pg + 1) * 2048)
                h0a = gps.tile([HID, 1024], F32, tag="gps", name=f"h0a{pg}")
                h0b = gps.tile([HID, 1024], F32, tag="gps", name=f"h0b{pg}")
                for k in range(4):
                    ek = wk.tile([128, 2048], F32, tag="embk")
                    nc.sync.dma_start(ek[:], embT[k * 128:(k + 1) * 128, csl])
                    for q in range(4):
                        dst = (h0a, h0b)[q // 2]
                        nc.tensor.matmul(
                            dst[:, (q % 2) * 512:(q % 2 + 1) * 512],
                            win_t[k][:],
                            ek[:, q * 512:(q + 1) * 512],
                            start=(k == 0), stop=(k == 3))
                for q in range(2):
                    src = (h0a, h0b)[q]
                    osl = slice(pg * 2048 + q * 1024, pg * 2048 + (q + 1) * 1024)
                    sg0 = wk.tile([HID, 1024], F32, tag="sg0")
                    nc.scalar.activation(sg0[:], src[:], AF.Sigmoid)
                    nc.vector.tensor_tensor(out=S[0:HID, osl], in0=src[:],
                                            in1=sg0[:], op=OP.mult)

            # ---- autoregressive loop
            for t in range(MAXLEN):
                gt = wk.tile([128, VW], F32, tag="gum")
                nc.sync.dma_start(gt[:], gum[t])

                # gate phase: h <- GRU(onehot, h), in n-tile pairs of 1024
                for p_ in range(NP):
                    hsl = slice(p_ * 1024, (p_ + 1) * 1024)
                    zr2 = gps.tile([128, 1024], F32, tag="gps")
                    hx2 = gps.tile([128, 1024], F32, tag="gps")
                    for q in range(2):
                        nt = p_ * 2 + q
                        sl = slice(nt * 512, (nt + 1) * 512)
                        nc.tensor.matmul(zr2[:, q * 512:(q + 1) * 512],
                                         wzr_s[:], S[:, sl],
                                         start=True, stop=True)
                        nc.tensor.matmul(hx2[:, q * 512:(q + 1) * 512],
                                         whx_s[:], S[:, sl],
                                         start=True, stop=True)
                    zrs = wk.tile([128, 1024], F32, tag="zrs")
                    nc.scalar.activation(zrs[:], zr2[:], AF.Sigmoid)
                    t1 = wk1.tile([HID, 1024], F32, tag="t1")
                    nc.vector.tensor_tensor(out=t1[:], in0=zrs[HID:128, :],
                                            in1=hx2[0:HID, :], op=OP.mult)
                    t2 = wk1.tile([HID, 1024], F32, tag="t2")
                    nc.vector.tensor_tensor(out=t2[:], in0=t1[:],
                                            in1=hx2[HID:128, :], op=OP.add)
                    nn_ = wk.tile([HID, 1024], F32, tag="nn")
                    nc.scalar.activation(nn_[:], t2[:], AF.Tanh)
                    dd = wk1.tile([HID, 1024], F32, tag="dd")
                    nc.vector.tensor_tensor(out=dd[:], in0=S[0:HID, hsl],
                                            in1=nn_[:], op=OP.subtract)
                    ee = wk1.tile([HID, 1024], F32, tag="ee")
                    nc.gpsimd.tensor_tensor(out=ee[:], in0=dd[:],
                                            in1=zrs[0:HID, :], op=OP.mult)
                    nc.gpsimd.tensor_tensor(out=S[0:HID, hsl], in0=ee[:],
                                            in1=nn_[:], op=OP.add)

                # logit phase: 32 groups of 128 rows, 4 per 1-bank psum tile
                L = wk.tile([128, VW], F32, tag="L")
                for bk in range(8):
                    lg = lps.tile([128, 4 * OUT], F32, tag="lps")
                    for g in range(4):
                        m = bk * 4 + g
                        nc.tensor.matmul(
                            lg[:, g * OUT:(g + 1) * OUT],
                            S[0:HID, m * 128:(m + 1) * 128],
                            wout_s[:], start=True, stop=True)
                    nc.vector.tensor_copy(
                        L[:, bk * 4 * OUT:(bk + 1) * 4 * OUT], lg[:])

                L3 = L[:].rearrange("p (m v) -> p m v", v=OUT)
                Lg = wk.tile([128, VW], F32, tag="Lg")
                Lg3 = Lg[:].rearrange("p (m v) -> p m v", v=OUT)
                nc.vector.tensor_tensor(out=Lg[:], in0=L[:], in1=gt[:], op=OP.add)

                mg = wk.tile([128, M], F32, tag="mg")
                nc.vector.tensor_reduce(mg[:], Lg3, axis=AX.X, op=OP.max)
                OH = wk.tile([128, VW], F32, tag="OH")
                OH3 = OH[:].rearrange("p (m v) -> p m v", v=OUT)
                nc.vector.tensor_tensor(
                    out=OH3, in0=Lg3,
                    in1=mg[:][:, :, None].broadcast_to([128, M, OUT]),
                    op=OP.is_equal)

                # --- critical path first: sample token -> bridge ---
                term_i = wk1.tile([128, M], I32, tag="term")
                nc.vector.tensor_copy(term_i[:], OH3[:, :, END:END + 1])
                SM = wk1.tile([128, VW], F32, tag="SM")
                SM3 = SM[:].rearrange("p (m v) -> p m v", v=OUT)
                nc.vector.tensor_tensor(
                    out=SM3, in0=OH3,
                    in1=c26[:][:, None, :].broadcast_to([128, M, OUT]),
                    op=OP.mult)
                tokcol = wk.tile([128, M], F32, tag="tokcol")
                nc.vector.tensor_reduce(tokcol[:], SM3, axis=AX.X, op=OP.add)
                nc.vector.copy_predicated(tokcol[:], done_i[:],
                                          c27t[:].broadcast_to([128, M]))
                if t < MAXLEN - 1:
                    tp = lps.tile([M, 128], F32, tag="lps")
                    nc.tensor.transpose(tp[:], tokcol[:], id_s[:])
                    tokT = wk.tile([M, 128], BF16, tag="tokT")
                    nc.vector.tensor_copy(tokT[:], tp[:])
                    nc.sync.dma_start(tokr2[0:1, :], tokT[:])
                    build_onehot()

                # --- off-path: softmax / p / outputs ---
                ms = wk1.tile([128, M], F32, tag="ms")
                nc.vector.tensor_reduce(ms[:], L3, axis=AX.X, op=OP.max)
                Lc = wk1.tile([128, VW], F32, tag="Lc")
                Lc3 = Lc[:].rearrange("p (m v) -> p m v", v=OUT)
                nc.vector.tensor_tensor(
                    out=Lc3, in0=L3,
                    in1=ms[:][:, :, None].broadcast_to([128, M, OUT]),
                    op=OP.subtract)
                E = wk.tile([128, VW], F32, tag="E")
                nc.scalar.activation(E[:], Lc[:], AF.Exp)
                E3 = E[:].rearrange("p (m v) -> p m v", v=OUT)
                sE = wk1.tile([128, M], F32, tag="sE")
                nc.vector.tensor_reduce(sE[:], E3, axis=AX.X, op=OP.add)
                rcp = wk1.tile([128, M], F32, tag="rcp")
                nc.vector.reciprocal(rcp[:], sE[:])
                # prob overwrites E in place
                nc.vector.tensor_tensor(
                    out=E3, in0=E3,
                    in1=rcp[:][:, :, None].broadcast_to([128, M, OUT]),
                    op=OP.mult)
                nc.scalar.dma_start(o_post[t], E[:])

                # p = prob[sample]
                PM = wk1.tile([128, VW], F32, tag="PM")
                nc.vector.tensor_tensor(out=PM[:], in0=E[:], in1=OH[:],
                                        op=OP.mult)
                PM3 = PM[:].rearrange("p (m v) -> p m v", v=OUT)
                pp = wk1.tile([128, M], F32, tag="pp")
                nc.vector.tensor_reduce(pp[:], PM3, axis=AX.X, op=OP.add)
                nc.vector.copy_predicated(pp[:], done_i[:],
                                          ones[:].broadcast_to([128, M]))
                nc.vector.tensor_tensor(out=done_i[:], in0=done_i[:],
                                        in1=term_i[:], op=OP.max)

                lp = wk1.tile([128, M], F32, tag="lp")
                nc.scalar.activation(lp[:], pp[:], AF.Ln)
                nc.vector.tensor_tensor(out=acc[:], in0=acc[:], in1=lp[:],
                                        op=OP.add)

                toks_i = wk.tile([128, M], I32, tag="toks_i")
                nc.vector.tensor_copy(toks_i[:], tokcol[:])
                nc.scalar.dma_start(o_toks[t], toks_i[:])
                nc.scalar.dma_start(o_masks[t], done_i[:])

            nc.sync.dma_start(o_logp[:], acc[:])

    nc.compile()
    return nc


_NC_CACHE = {}


def _get_nc():
    if "nc" not in _NC_CACHE:
        _NC_CACHE["nc"] = _build_nc()
    return _NC_CACHE["nc"]


def _host_prep(inputs):
    emb = np.ascontiguousarray(np.asarray(inputs["embedding"], np.float32))
    embed_table = np.asarray(inputs["embed_table"], np.float32)
    W_in = np.ascontiguousarray(np.asarray(inputs["W_in"], np.float32))
    b_in = np.asarray(inputs["b_in"], np.float32)
    Wx = np.asarray(inputs["Wx"], np.float32)
    Wh = np.asarray(inputs["Wh"], np.float32)
    bx = np.asarray(inputs["bx"], np.float32)
    bh = np.asarray(inputs["bh"], np.float32)
    W_out = np.ascontiguousarray(np.asarray(inputs["W_out"], np.float32))
    b_out = np.asarray(inputs["b_out"], np.float32)

    if np.abs(b_in).max() > 0 or np.abs(bh).max() > 0 or np.abs(b_out).max() > 0:
        # biases are folded assuming the zero-initialized reference setup
        raise NotImplementedError("nonzero b_in/bh/b_out not supported")

    xa_table = (embed_table @ Wx + bx).astype(np.float32)   # (28, 192)

    wzr = np.zeros((KG, 128), np.float32)
    wzr[0:HID, :] = Wh[:, 0:128]
    wzr[HID:KG, :] = xa_table[:, 0:128]
    whx = np.zeros((KG, 128), np.float32)
    whx[0:HID, 0:HID] = Wh[:, 128:192]          # hh -> out partitions 0:64
    whx[HID:KG, HID:128] = xa_table[:, 128:192]  # xh -> out partitions 64:128

    # bridge lhsT: psum[v, b] = tok[b]*1 + 1*(-v)
    wbr = np.zeros((2, PAD + 1), np.float32)
    wbr[0, :] = 1.0
    wbr[1, :] = -np.arange(PAD + 1, dtype=np.float32)

    # gumbel noise, bit-exact with jax.random.categorical on CPU
    import jax
    import jax.numpy as jnp
    cpu = jax.devices("cpu")[0]
    with jax.default_device(cpu):
        keys = jax.random.split(jax.random.key(42), MAXLEN)
        G = np.stack([
            np.asarray(jax.random.gumbel(keys[t], (BATCH, OUT), jnp.float32))
            for t in range(MAXLEN)])                      # (25, 32768, 26)

    ident = np.eye(128, dtype=np.float32)

    in_maps = []
    for c in range(NCORES):
        off = c * B
        embT = np.ascontiguousarray(emb[off:off + B, :].T)          # (512, 4096)
        gc = G[:, off:off + B, :].reshape(MAXLEN, M, 128, OUT)
        gc = np.ascontiguousarray(gc.transpose(0, 2, 1, 3).reshape(MAXLEN, 128, VW))
        in_maps.append({
            "embT": embT, "gum": gc, "wzr": wzr, "whx": whx,
            "w_in": W_in, "wout": W_out, "wbr": wbr, "ident": ident,
        })
    return in_maps


def _assemble(results):
    msg_parts, logp_parts, post_parts, mask_parts = [], [], [], []
    for c in range(NCORES):
        r = results[c]
        toks = r["o_toks"].transpose(0, 2, 1).reshape(MAXLEN, B)     # [t, b]
        masks = r["o_masks"].transpose(0, 2, 1).reshape(MAXLEN, B)
        logp = r["o_logp"].transpose(1, 0).reshape(B)
        post = (r["o_post"].reshape(MAXLEN, 128, M, OUT)
                .transpose(2, 1, 0, 3).reshape(B, MAXLEN, OUT))
        done_fin = masks[-1]
        final = (END + 2 * done_fin).astype(np.int32)
        msg = np.concatenate([
            np.full((B, 1), START, np.int32),
            toks.T.astype(np.int32),
            final[:, None]], axis=1)
        msg_parts.append(msg)
        logp_parts.append(logp.astype(np.float32))
        post_parts.append(post[:, :, None, :].astype(np.float32))
        mask_parts.append(masks.T[:, :, None].astype(np.int32))
    return (np.concatenate(msg_parts, 0), np.concatenate(logp_parts, 0),
            np.concatenate(post_parts, 0), np.concatenate(mask_parts, 0))


def kernel(**inputs):
    nc = _get_nc()
    in_maps = _host_prep(inputs)
    res = run_bass_kernel_spmd(nc, in_maps, list(range(NCORES)))
    return _assemble(res.results)


if __name__ == "__main__":
    d = np.load("/tmp/inputs.npz")
    inputs = {k: d[k] for k in d.files}
    out = kernel(**inputs)
    for name, a in zip(["message", "logits", "posterior", "masks"], out):
        print(name, a.shape, a.dtype)


# revision 19
# speedup vs baseline: 1.3509x; 1.3195x over previous
"""Trainium2 Bass kernel for nn_ActorCom (GRU autoregressive sampler).

Contract: kernel(**inputs) takes FULL unsharded inputs (keyed as in
reference.setup_inputs()) and returns the FULL output tuple
(message, logits, posterior, masks) matching reference.reference(**inputs).

Strategy (pure data parallel over 8 NeuronCores, 4096 batch rows each):
  - jax.random.categorical(key, logit) == argmax(logit + gumbel(key)); the keys
    are compile-time constants (jax.random.key(42)), so the gumbel noise is
    input-independent and precomputed on host (CPU jax, bit-exact with the
    reference) and streamed to the device.
  - The embedding gather xa = (embed_table @ Wx + bx)[token] has only 28
    possible rows; it is realized as a one-hot (28,B) matmul fused into the
    gate matmul via a stacked contraction of K=92 = 64 (h) + 28 (one-hot).
  - State layout: S (92, 4096) = [hT (64,B); one-hotT (28,B)].  Gate
    pre-activations come out gate-major: one K=92 M=128 N=512 fp32 matmul per
    512-batch tile for [z|r], one for [hh|xh] (block-diagonal lhsT).
  - Sampling runs batch-major: logits (128,26) per 128-row group via
    lhsT=S-slice matmuls; argmax via segmented max + is_equal one-hot; softmax
    with segmented reductions; token/done updates with predicated copies.
  - Bridge to the next step: sampled tokens (128,32) are PE-transposed,
    DMA-linearized to a (1,4096) bf16 row, expanded to (28,B) differences
    tok[b]-v via a K=2 bf16 matmul against [ones; -iota28], and compared
    against 0 to rebuild one-hotT (exact: all values are small integers).
All gate/logit matmuls are fp32 (fp32r measured 1.9e-4 scale-relative error
on HW - too coarse for stable argmax; fp32 measured 2.7e-7).
"""
import numpy as np

import concourse.bass as bass
import concourse.mybir as mybir
import concourse.tile as tile
from concourse import bacc
from concourse.bass_utils import run_bass_kernel_spmd

F32 = mybir.dt.float32
BF16 = mybir.dt.bfloat16
I32 = mybir.dt.int32
AF = mybir.ActivationFunctionType
OP = mybir.AluOpType
AX = mybir.AxisListType

VOCAB = 25
END, START, PAD = 25, 26, 27
EMBED, HID, OUT, MAXLEN = 3, 64, 26, 25
BATCH, FEAT = 32768, 512
NCORES = 8
B = BATCH // NCORES          # 4096 per core
M = B // 128                 # 32 m-groups of 128 rows
KG = HID + PAD + 1           # 92 = 64 + 28 stacked contraction
VW = M * OUT                 # 832 = 32 groups * 26
NP = B // 1024               # 4 n-tile pairs of 1024


def _build_nc():
    nc = bacc.Bacc("TRN2", target_bir_lowering=False, debug=False,
                   num_devices=NCORES)

    embT = nc.dram_tensor("embT", [FEAT, B], F32, kind="ExternalInput")
    gum = nc.dram_tensor("gum", [MAXLEN, 128, VW], F32, kind="ExternalInput")
    wzr = nc.dram_tensor("wzr", [KG, 128], F32, kind="ExternalInput")
    whx = nc.dram_tensor("whx", [KG, 128], F32, kind="ExternalInput")
    w_in = nc.dram_tensor("w_in", [FEAT, HID], F32, kind="ExternalInput")
    wout = nc.dram_tensor("wout", [HID, OUT], F32, kind="ExternalInput")
    wbr = nc.dram_tensor("wbr", [2, PAD + 1], F32, kind="ExternalInput")
    ident = nc.dram_tensor("ident", [128, 128], F32, kind="ExternalInput")

    o_toks = nc.dram_tensor("o_toks", [MAXLEN, 128, M], I32, kind="ExternalOutput")
    o_masks = nc.dram_tensor("o_masks", [MAXLEN, 128, M], I32, kind="ExternalOutput")
    o_logp = nc.dram_tensor("o_logp", [128, M], F32, kind="ExternalOutput")
    o_post = nc.dram_tensor("o_post", [MAXLEN, 128, VW], F32, kind="ExternalOutput")

    with tile.TileContext(nc) as tc:
        with (
            tc.tile_pool(name="wp", bufs=1) as wp,
            tc.tile_pool(name="st", bufs=1) as st,
            tc.tile_pool(name="wk", bufs=2) as wk,
            tc.tile_pool(name="wk1", bufs=1) as wk1,
            tc.tile_pool(name="gps", bufs=3, space="PSUM") as gps,
            tc.tile_pool(name="lps", bufs=2, space="PSUM") as lps,
        ):
            # ---- persistent weights / constants
            wzr_s = wp.tile([KG, 128], F32, tag="wzr")
            whx_s = wp.tile([KG, 128], F32, tag="whx")
            wout_s = wp.tile([HID, OUT], F32, tag="wout")
            id_s = wp.tile([128, 128], F32, tag="ident")
            wbr_f = wp.tile([2, PAD + 1], F32, tag="wbrf")
            wbr_s = wp.tile([2, PAD + 1], BF16, tag="wbr")
            nc.sync.dma_start(wzr_s[:], wzr[:])
            nc.sync.dma_start(whx_s[:], whx[:])
            nc.sync.dma_start(wout_s[:], wout[:])
            nc.sync.dma_start(id_s[:], ident[:])
            nc.sync.dma_start(wbr_f[:], wbr[:])
            nc.vector.tensor_copy(wbr_s[:], wbr_f[:])

            c26 = wp.tile([128, OUT], F32, tag="c26")
            nc.gpsimd.iota(c26[:], pattern=[[1, OUT]], base=0,
                           channel_multiplier=0,
                           allow_small_or_imprecise_dtypes=True)
            c27t = wp.tile([128, 1], F32, tag="c27t")
            nc.vector.memset(c27t[:], float(PAD))
            ones = wp.tile([128, 1], F32, tag="ones")
            nc.vector.memset(ones[:], 1.0)

            # bridge rhs: row 0 = token row (bf16, refreshed), row 1 = ones
            tokr2 = st.tile([2, B], BF16, tag="tokr2")
            nc.vector.memset(tokr2[0:2, :], 1.0)   # row 1 stays ones forever

            # ---- persistent state
            S = st.tile([KG, B], F32, tag="S")
            done_i = st.tile([128, M], I32, tag="done")
            acc = st.tile([128, M], F32, tag="acc")
            nc.vector.memset(done_i[:], 0)
            nc.vector.memset(acc[:], 0.0)

            def build_onehot(jlo, jhi):
                """S[64:92, 512*jlo:512*jhi] <- onehot from tokr2[0] (K=2 MM)."""
                for j in range(jlo, jhi):
                    sl = slice(j * 512, (j + 1) * 512)
                    dp = lps.tile([PAD + 1, 512], F32, tag="lps", name=f"dp{j}")
                    nc.tensor.matmul(dp[:], wbr_s[:], tokr2[:, sl],
                                     start=True, stop=True)
                    nc.vector.tensor_scalar(
                        out=S[HID:KG, sl], in0=dp[:], scalar1=0.0,
                        scalar2=None, op0=OP.is_equal)

            # one-hot(START) for step 0
            nc.vector.memset(tokr2[0:1, :], float(START))
            build_onehot(0, 8)

            # ---- h0 = silu(emb @ W_in) ; computed transposed into S[0:64]
            win_t = [wp.tile([128, HID], F32, tag=f"win{k}", name=f"win{k}")
                     for k in range(4)]
            for k in range(4):
                nc.sync.dma_start(win_t[k][:], w_in[k * 128:(k + 1) * 128, :])
            for pg in range(2):          # pair-groups of 2048 batch cols
                csl = slice(pg * 2048, (pg + 1) * 2048)
                h0a = gps.tile([HID, 1024], F32, tag="gps", name=f"h0a{pg}")
                h0b = gps.tile([HID, 1024], F32, tag="gps", name=f"h0b{pg}")
                for k in range(4):
                    ek = wk.tile([128, 2048], F32, tag="embk")
                    nc.sync.dma_start(ek[:], embT[k * 128:(k + 1) * 128, csl])
                    for q in range(4):
                        dst = (h0a, h0b)[q // 2]
                        nc.tensor.matmul(
                            dst[:, (q % 2) * 512:(q % 2 + 1) * 512],
                            win_t[k][:],
                            ek[:, q * 512:(q + 1) * 512],
                            start=(k == 0), stop=(k == 3))
                for q in range(2):
                    src = (h0a, h0b)[q]
                    osl = slice(pg * 2048 + q * 1024, pg * 2048 + (q + 1) * 1024)
                    sg0 = wk.tile([HID, 1024], F32, tag="sg0")
                    nc.scalar.activation(sg0[:], src[:], AF.Sigmoid)
                    nc.vector.tensor_tensor(out=S[0:HID, osl], in0=src[:],
                                            in1=sg0[:], op=OP.mult)

            # ---- autoregressive loop
            for t in range(MAXLEN):
                gt = wk.tile([128, VW], F32, tag="gum")
                nc.sync.dma_start(gt[:], gum[t])

                # gate phase: h <- GRU(onehot, h), in n-tile pairs of 1024
                for p_ in range(NP):
                    hsl = slice(p_ * 1024, (p_ + 1) * 1024)
                    zr2 = gps.tile([128, 1024], F32, tag="gps")
                    hx2 = gps.tile([128, 1024], F32, tag="gps")
                    for q in range(2):
                        nt = p_ * 2 + q
                        sl = slice(nt * 512, (nt + 1) * 512)
                        nc.tensor.matmul(zr2[:, q * 512:(q + 1) * 512],
                                         wzr_s[:], S[:, sl],
                                         start=True, stop=True)
                        nc.tensor.matmul(hx2[:, q * 512:(q + 1) * 512],
                                         whx_s[:], S[:, sl],
                                         start=True, stop=True)
                    zrs = wk.tile([128, 1024], F32, tag="zrs")
                    nc.scalar.activation(zrs[:], zr2[:], AF.Sigmoid)
                    t1 = wk.tile([HID, 1024], F32, tag="t1")
                    nc.vector.tensor_tensor(out=t1[:], in0=zrs[HID:128, :],
                                            in1=hx2[0:HID, :], op=OP.mult)
                    t2 = wk.tile([HID, 1024], F32, tag="t2")
                    nc.vector.tensor_tensor(out=t2[:], in0=t1[:],
                                            in1=hx2[HID:128, :], op=OP.add)
                    nn_ = wk.tile([HID, 1024], F32, tag="nn")
                    nc.scalar.activation(nn_[:], t2[:], AF.Tanh)
                    dd = wk.tile([HID, 1024], F32, tag="dd")
                    nc.gpsimd.tensor_tensor(out=dd[:], in0=S[0:HID, hsl],
                                            in1=nn_[:], op=OP.subtract)
                    ee = wk.tile([HID, 1024], F32, tag="ee")
                    nc.gpsimd.tensor_tensor(out=ee[:], in0=dd[:],
                                            in1=zrs[0:HID, :], op=OP.mult)
                    nc.gpsimd.tensor_tensor(out=S[0:HID, hsl], in0=ee[:],
                                            in1=nn_[:], op=OP.add)

                # logit phase + critical token path, in halves of 16 m-groups
                # so the bridge for batch 0:2048 runs while 2048:4096 samples
                L = wk.tile([128, VW], F32, tag="L")
                L3 = L[:].rearrange("p (m v) -> p m v", v=OUT)
                Lg = wk.tile([128, VW], F32, tag="Lg")
                Lg3 = Lg[:].rearrange("p (m v) -> p m v", v=OUT)
                OH = wk.tile([128, VW], F32, tag="OH")
                OH3 = OH[:].rearrange("p (m v) -> p m v", v=OUT)
                mg = wk.tile([128, M], F32, tag="mg")
                term_i = wk1.tile([128, M], I32, tag="term")
                SM = wk1.tile([128, VW], F32, tag="SM")
                SM3 = SM[:].rearrange("p (m v) -> p m v", v=OUT)
                tokcol = wk.tile([128, M], F32, tag="tokcol")
                HM = M // 2
                HW_ = HM * OUT  # 416 columns per half
                for sh in range(2):
                    msl = slice(sh * HM, (sh + 1) * HM)
                    csl = slice(sh * HW_, (sh + 1) * HW_)
                    for bk in range(sh * 4, sh * 4 + 4):
                        lg = lps.tile([128, 4 * OUT], F32, tag="lps")
                        for g in range(4):
                            m = bk * 4 + g
                            nc.tensor.matmul(
                                lg[:, g * OUT:(g + 1) * OUT],
                                S[0:HID, m * 128:(m + 1) * 128],
                                wout_s[:], start=True, stop=True)
                        nc.vector.tensor_copy(
                            L[:, bk * 4 * OUT:(bk + 1) * 4 * OUT], lg[:])
                    nc.vector.tensor_tensor(out=Lg[:, csl], in0=L[:, csl],
                                            in1=gt[:, csl], op=OP.add)
                    nc.vector.tensor_reduce(mg[:, msl], Lg3[:, msl, :],
                                            axis=AX.X, op=OP.max)
                    nc.vector.tensor_tensor(
                        out=OH3[:, msl, :], in0=Lg3[:, msl, :],
                        in1=mg[:, msl][:, :, None].broadcast_to([128, HM, OUT]),
                        op=OP.is_equal)
                    nc.vector.tensor_copy(term_i[:, msl],
                                          OH3[:, msl, END:END + 1])
                    nc.vector.tensor_tensor(
                        out=SM3[:, msl, :], in0=OH3[:, msl, :],
                        in1=c26[:][:, None, :].broadcast_to([128, HM, OUT]),
                        op=OP.mult)
                    nc.vector.tensor_reduce(tokcol[:, msl], SM3[:, msl, :],
                                            axis=AX.X, op=OP.add)
                    nc.vector.copy_predicated(
                        tokcol[:, msl], done_i[:, msl],
                        c27t[:].broadcast_to([128, HM]))
                    if t < MAXLEN - 1:
                        tp = lps.tile([HM, 128], F32, tag="lps")
                        nc.tensor.transpose(tp[:], tokcol[:, msl], id_s[:])
                        tokT = wk.tile([HM, 128], BF16, tag="tokT")
                        nc.vector.tensor_copy(tokT[:], tp[:])
                        nc.sync.dma_start(
                            tokr2[0:1, sh * 2048:(sh + 1) * 2048], tokT[:])
                        build_onehot(sh * 4, sh * 4 + 4)

                # --- off-path: softmax / p / outputs ---
                ms = wk1.tile([128, M], F32, tag="ms")
                nc.vector.tensor_reduce(ms[:], L3, axis=AX.X, op=OP.max)
                Lc = wk1.tile([128, VW], F32, tag="Lc")
                Lc3 = Lc[:].rearrange("p (m v) -> p m v", v=OUT)
                nc.vector.tensor_tensor(
                    out=Lc3, in0=L3,
                    in1=ms[:][:, :, None].broadcast_to([128, M, OUT]),
                    op=OP.subtract)
                E = wk.tile([128, VW], F32, tag="E")
                nc.scalar.activation(E[:], Lc[:], AF.Exp)
                E3 = E[:].rearrange("p (m v) -> p m v", v=OUT)
                sE = wk1.tile([128, M], F32, tag="sE")
                nc.vector.tensor_reduce(sE[:], E3, axis=AX.X, op=OP.add)
                rcp = wk1.tile([128, M], F32, tag="rcp")
                nc.vector.reciprocal(rcp[:], sE[:])
                # prob overwrites E in place
                nc.vector.tensor_tensor(
                    out=E3, in0=E3,
                    in1=rcp[:][:, :, None].broadcast_to([128, M, OUT]),
                    op=OP.mult)
                nc.scalar.dma_start(o_post[t], E[:])

                # p = prob[sample]
                PM = wk1.tile([128, VW], F32, tag="PM")
                nc.vector.tensor_tensor(out=PM[:], in0=E[:], in1=OH[:],
                                        op=OP.mult)
                PM3 = PM[:].rearrange("p (m v) -> p m v", v=OUT)
                pp = wk1.tile([128, M], F32, tag="pp")
                nc.vector.tensor_reduce(pp[:], PM3, axis=AX.X, op=OP.add)
                nc.vector.copy_predicated(pp[:], done_i[:],
                                          ones[:].broadcast_to([128, M]))
                nc.vector.tensor_tensor(out=done_i[:], in0=done_i[:],
                                        in1=term_i[:], op=OP.max)

                lp = wk1.tile([128, M], F32, tag="lp")
                nc.scalar.activation(lp[:], pp[:], AF.Ln)
                nc.vector.tensor_tensor(out=acc[:], in0=acc[:], in1=lp[:],
                                        op=OP.add)

                toks_i = wk.tile([128, M], I32, tag="toks_i")
                nc.vector.tensor_copy(toks_i[:], tokcol[:])
                nc.scalar.dma_start(o_toks[t], toks_i[:])
                nc.scalar.dma_start(o_masks[t], done_i[:])

            nc.sync.dma_start(o_logp[:], acc[:])

    nc.compile()
    return nc


_NC_CACHE = {}


def _get_nc():
    if "nc" not in _NC_CACHE:
        _NC_CACHE["nc"] = _build_nc()
    return _NC_CACHE["nc"]


def _host_prep(inputs):
    emb = np.ascontiguousarray(np.asarray(inputs["embedding"], np.float32))
    embed_table = np.asarray(inputs["embed_table"], np.float32)
    W_in = np.ascontiguousarray(np.asarray(inputs["W_in"], np.float32))
    b_in = np.asarray(inputs["b_in"], np.float32)
    Wx = np.asarray(inputs["Wx"], np.float32)
    Wh = np.asarray(inputs["Wh"], np.float32)
    bx = np.asarray(inputs["bx"], np.float32)
    bh = np.asarray(inputs["bh"], np.float32)
    W_out = np.ascontiguousarray(np.asarray(inputs["W_out"], np.float32))
    b_out = np.asarray(inputs["b_out"], np.float32)

    if np.abs(b_in).max() > 0 or np.abs(bh).max() > 0 or np.abs(b_out).max() > 0:
        # biases are folded assuming the zero-initialized reference setup
        raise NotImplementedError("nonzero b_in/bh/b_out not supported")

    xa_table = (embed_table @ Wx + bx).astype(np.float32)   # (28, 192)

    wzr = np.zeros((KG, 128), np.float32)
    wzr[0:HID, :] = Wh[:, 0:128]
    wzr[HID:KG, :] = xa_table[:, 0:128]
    whx = np.zeros((KG, 128), np.float32)
    whx[0:HID, 0:HID] = Wh[:, 128:192]          # hh -> out partitions 0:64
    whx[HID:KG, HID:128] = xa_table[:, 128:192]  # xh -> out partitions 64:128

    # bridge lhsT: psum[v, b] = tok[b]*1 + 1*(-v)
    wbr = np.zeros((2, PAD + 1), np.float32)
    wbr[0, :] = 1.0
    wbr[1, :] = -np.arange(PAD + 1, dtype=np.float32)

    # gumbel noise, bit-exact with jax.random.categorical on CPU
    import jax
    import jax.numpy as jnp
    cpu = jax.devices("cpu")[0]
    with jax.default_device(cpu):
        keys = jax.random.split(jax.random.key(42), MAXLEN)
        G = np.stack([
            np.asarray(jax.random.gumbel(keys[t], (BATCH, OUT), jnp.float32))
            for t in range(MAXLEN)])                      # (25, 32768, 26)

    ident = np.eye(128, dtype=np.float32)

    in_maps = []
    for c in range(NCORES):
        off = c * B
        embT = np.ascontiguousarray(emb[off:off + B, :].T)          # (512, 4096)
        gc = G[:, off:off + B, :].reshape(MAXLEN, M, 128, OUT)
        gc = np.ascontiguousarray(gc.transpose(0, 2, 1, 3).reshape(MAXLEN, 128, VW))
        in_maps.append({
            "embT": embT, "gum": gc, "wzr": wzr, "whx": whx,
            "w_in": W_in, "wout": W_out, "wbr": wbr, "ident": ident,
        })
    return in_maps


def _assemble(results):
    msg_parts, logp_parts, post_parts, mask_parts = [], [], [], []
    for c in range(NCORES):
        r = results[c]
        toks = r["o_toks"].transpose(0, 2, 1).reshape(MAXLEN, B)     # [t, b]
        masks = r["o_masks"].transpose(0, 2, 1).reshape(MAXLEN, B)
        logp = r["o_logp"].transpose(1, 0).reshape(B)
        post = (r["o_post"].reshape(MAXLEN, 128, M, OUT)
                .transpose(2, 1, 0, 3).reshape(B, MAXLEN, OUT))
        done_fin = masks[-1]
        final = (END + 2 * done_fin).astype(np.int32)
        msg = np.concatenate([
            np.full((B, 1), START, np.int32),
            toks.T.astype(np.int32),
            final[:, None]], axis=1)
        msg_parts.append(msg)
        logp_parts.append(logp.astype(np.float32))
        post_parts.append(post[:, :, None, :].astype(np.float32))
        mask_parts.append(masks.T[:, :, None].astype(np.int32))
    return (np.concatenate(msg_parts, 0), np.concatenate(logp_parts, 0),
            np.concatenate(post_parts, 0), np.concatenate(mask_parts, 0))


def kernel(**inputs):
    nc = _get_nc()
    in_maps = _host_prep(inputs)
    res = run_bass_kernel_spmd(nc, in_maps, list(range(NCORES)))
    return _assemble(res.results)


if __name__ == "__main__":
    d = np.load("/tmp/inputs.npz")
    inputs = {k: d[k] for k in d.files}
    out = kernel(**inputs)
    for name, a in zip(["message", "logits", "posterior", "masks"], out):
        print(name, a.shape, a.dtype)
